# revision 5
# baseline (speedup 1.0000x reference)
"""DotPredictor on 8 Trainium2 NeuronCores.

score[e] = <h[src[e]], h[dst[e]]> ; edge_index [2, 600000] int64, h [100000,128] f32.

Strategy (data-parallel over edges, h replicated per core):
  - 75,000 edges per core.
  - Rows of h are gathered with the custom SWDGE `dma_gather` instruction
    (int16 indices). Since 100k rows exceed int16, node rows are split into
    4 buckets of 25,000 rows; each core's edges are grouped on the host by
    (src_bucket, dst_bucket) -> 16 groups, each gathered against the right
    h base offset with bucket-local indices.
  - Gathers are issued in 1024-index chunks (the SWDGE descriptor ring caps
    at ~65 descs/engine), round-robin across the 4 SWDGE queues so all four
    GPSIMD Q7 core-pairs generate descriptors in parallel -- measured
    ~364 GB/s sustained random-row gather per core (HBM roofline).
  - Each chunk pair (src rows, dst rows; [128, 8, 128] f32 tiles, edge
    (tile-col, partition) layout) is multiplied and row-reduced on DVE into
    a scores tile, stored contiguously at the end.
  - Host maps scores back through the per-core group sort permutation.

Group sizes vary per core; since the program is shared (SPMD), chunks are
padded to the static capacity with a ramp of distinct valid row indices
(descriptor counts must match the static num_idxs_reg -- a mismatch drifts
the SWDGE ring bookkeeping; and a constant pad row would hammer one DRAM
row). Padded lanes are discarded by the host-side inverse mapping.
"""

import ml_dtypes
import numpy as np

import concourse.bacc as bacc
import concourse.mybir as mybir
import concourse.tile as tile

N_CORES = 8
N_NODES = 100000
N_EDGES = 600000
D = 128
P = 128

E_PER_CORE = N_EDGES // N_CORES      # 75000
N_BUCKETS = 4
BUCKET = 25000                        # int16-safe local indices
N_GROUPS = N_BUCKETS * N_BUCKETS      # 16
K_GATHER = 1024                       # idxs per dma_gather (ring-safe)
CHUNK_TILES = K_GATHER // P           # 8 tiles of 128 edges per gather
N_QUEUES = 4
GATH_BUFS = 16                        # SBUF slots for gather tiles


def plan(all_src, all_dst):
    """Compute the shared static layout from the actual inputs.

    Returns (chunks_per_group [16], per-core group order/permutations).
    """
    per_core = []
    max_group = np.zeros(N_GROUPS, dtype=np.int64)
    for c in range(N_CORES):
        lo = c * E_PER_CORE
        src = all_src[lo:lo + E_PER_CORE]
        dst = all_dst[lo:lo + E_PER_CORE]
        gid = (src // BUCKET) * N_BUCKETS + dst // BUCKET
        order = np.argsort(gid, kind="stable")
        sizes = np.bincount(gid, minlength=N_GROUPS)
        max_group = np.maximum(max_group, sizes)
        per_core.append((src, dst, order, sizes))
    chunk_sizes = []
    for g in range(N_GROUPS):
        mg = int(max_group[g])
        full = mg // K_GATHER
        tail = mg - full * K_GATHER
        sizes_g = [K_GATHER] * full
        # tail rounded to 128-idx granularity (>=128) to cut pad traffic
        sizes_g.append(max(P, -(-tail // P) * P))
        chunk_sizes.append(sizes_g)
    return chunk_sizes, per_core


def build_program(chunks_per_group, repeats=1, do_dve=True, do_gather=True):
    all_chunks = [k for g in chunks_per_group for k in g]
    g_tot = sum(k // P for k in all_chunks)        # scores columns
    idx_cols = 2 * sum(k // 16 for k in all_chunks)

    nc = bacc.Bacc("TRN2", target_bir_lowering=False, debug=False,
                   num_swdge_queues=N_QUEUES)
    # bf16 h: halves random-gather traffic (256 B rows); rel-err budget 2e-2
    # dwarfs the ~0.4% bf16 product rounding.
    h = nc.dram_tensor("h", [N_NODES, D], mybir.dt.bfloat16,
                       kind="ExternalInput")
    idx = nc.dram_tensor("idx", [P, idx_cols], mybir.dt.int16,
                         kind="ExternalInput")
    out = nc.dram_tensor("scores", [P, g_tot], mybir.dt.float32,
                         kind="ExternalOutput")

    with tile.TileContext(nc) as tc:
        with (
            tc.tile_pool(name="idxp", bufs=1) as idx_pool,
            tc.tile_pool(name="sc", bufs=1) as sc_pool,
            tc.tile_pool(name="gp", bufs=GATH_BUFS) as gpool,
        ):
            idx_t = idx_pool.tile([P, idx_cols], mybir.dt.int16)
            nc.sync.dma_start(out=idx_t[:], in_=idx[:])
            scores = sc_pool.tile([P, g_tot], mybir.dt.float32)
            if not do_dve:
                nc.vector.memset(scores[:], 0.0)

            for _rep in range(repeats):
              gath_i, col0, idx0 = 0, 0, 0
              for g in range(N_GROUPS):
                bs, bd = divmod(g, N_BUCKETS)
                h_src = h[bs * BUCKET:(bs + 1) * BUCKET, :]
                h_dst = h[bd * BUCKET:(bd + 1) * BUCKET, :]
                for kk in chunks_per_group[g]:
                    ct = kk // P                    # tiles this chunk
                    S = kk // 16                    # idx cols this chunk
                    sidx0 = idx0
                    didx0 = idx0 + S
                    s_full = gpool.tile([P, CHUNK_TILES, D],
                                        mybir.dt.bfloat16, tag="s")
                    d_full = gpool.tile([P, CHUNK_TILES, D],
                                        mybir.dt.bfloat16, tag="d")
                    s_t = s_full[:, :ct, :]
                    d_t = d_full[:, :ct, :]
                    if do_gather:
                        nc.gpsimd.dma_gather(
                            out_ap=s_t[:], in_ap=h_src,
                            idxs_ap=idx_t[:, sidx0:sidx0 + S],
                            num_idxs=kk, num_idxs_reg=kk,
                            elem_size=D, queue_num=(2 * gath_i) % N_QUEUES)
                        nc.gpsimd.dma_gather(
                            out_ap=d_t[:], in_ap=h_dst,
                            idxs_ap=idx_t[:, didx0:didx0 + S],
                            num_idxs=kk, num_idxs_reg=kk,
                            elem_size=D, queue_num=(2 * gath_i + 1) % N_QUEUES)
                    if not do_gather:
                        nc.gpsimd.memset(s_t[:], 0.0)
                        nc.gpsimd.memset(d_t[:], 0.0)
                    if do_dve:
                        nc.vector.tensor_mul(out=s_t[:], in0=s_t[:], in1=d_t[:])
                        nc.vector.tensor_reduce(
                            out=scores[:, col0:col0 + ct],
                            in_=s_t[:],
                            axis=mybir.AxisListType.X,
                            op=mybir.AluOpType.add)
                    gath_i += 1
                    col0 += ct
                    idx0 += 2 * S

            nc.sync.dma_start(out=out[:], in_=scores[:])
    nc.compile()
    return nc


def _wrap_block(flat_i16):
    """[k] int16 -> [128, k/16] wrapped (idx j at [j%16, j//16]), replicated
    across the 8 groups of 16 partitions (each SWDGE queue pair reads its
    own)."""
    k = flat_i16.shape[0]
    w = flat_i16.reshape(k // 16, 16).T               # [16, k/16]
    return np.ascontiguousarray(np.tile(w, (8, 1)))   # [128, k/16]


def make_core_inputs(src, dst, order, sizes, chunks_per_group, h,
                     pad_value=0):
    """Build one core's idx tensor + the (p, col) mapping for its edges."""
    cap = np.array([sum(g) for g in chunks_per_group])  # idx capacity/group
    total = int(cap.sum())
    src_s = src[order]
    dst_s = dst[order]

    h = np.asarray(h, dtype=ml_dtypes.bfloat16)

    # spread pad slots over distinct rows -- a constant pad row would
    # hammer one DRAM row/bank and stall the SDMA engines
    ramp = (np.arange(total, dtype=np.int64) * 997) % BUCKET
    src_flat = ramp.copy()
    dst_flat = ramp.copy()
    gstart_e = np.concatenate([[0], np.cumsum(sizes)])      # edges
    gstart_c = np.concatenate([[0], np.cumsum(cap)])        # idx slots
    tile_start = np.concatenate(
        [[0], np.cumsum([sum(k // P for k in g) for g in chunks_per_group])])

    p_arr = np.empty(E_PER_CORE, dtype=np.int64)
    col_arr = np.empty(E_PER_CORE, dtype=np.int64)
    for g in range(N_GROUPS):
        n_g = int(sizes[g])
        e0, c0 = int(gstart_e[g]), int(gstart_c[g])
        bs, bd = divmod(g, N_BUCKETS)
        src_flat[c0:c0 + n_g] = src_s[e0:e0 + n_g] - bs * BUCKET
        dst_flat[c0:c0 + n_g] = dst_s[e0:e0 + n_g] - bd * BUCKET
        j = np.arange(n_g)
        p_arr[e0:e0 + n_g] = j % P
        col_arr[e0:e0 + n_g] = tile_start[g] + j // P

    # interleave [src_chunk, dst_chunk] wrapped segments per gather
    segs = []
    off = 0
    for g in chunks_per_group:
        for kk in g:
            segs.append(_wrap_block(src_flat[off:off + kk].astype(np.int16)))
            segs.append(_wrap_block(dst_flat[off:off + kk].astype(np.int16)))
            off += kk
    idx_np = np.concatenate(segs, axis=1)
    return ({"h": h, "idx": np.ascontiguousarray(idx_np)},
            (order, p_arr, col_arr))


def run(edge_index, h, pad_value=0):
    from concourse.bass_utils import run_bass_kernel_spmd

    h = np.ascontiguousarray(np.asarray(h), dtype=np.float32)
    all_src = np.asarray(edge_index[0], dtype=np.int64)
    all_dst = np.asarray(edge_index[1], dtype=np.int64)
    chunks_per_group, per_core = plan(all_src, all_dst)
    nc = build_program(chunks_per_group)

    in_maps, mappings = [], []
    for c in range(N_CORES):
        src, dst, order, sizes = per_core[c]
        m, mapping = make_core_inputs(src, dst, order, sizes,
                                      chunks_per_group, h, pad_value)
        in_maps.append(m)
        mappings.append(mapping)

    res = run_bass_kernel_spmd(nc, in_maps, core_ids=list(range(N_CORES)))

    out = np.empty(N_EDGES, dtype=np.float32)
    for c in range(N_CORES):
        order, p_arr, col_arr = mappings[c]
        scores = res.results[c]["scores"]
        vals = scores[p_arr, col_arr]
        out[c * E_PER_CORE + order] = vals
    return out, res


def kernel(edge_index, h):
    out, _ = run(edge_index, h)
    return out



# revision 7
# speedup vs baseline: 1.0817x; 1.0817x over previous
"""DotPredictor on 8 TRN2 cores — src-run dedup over dma_gather.

score[e] = <h[src[e]], h[dst[e]]>.

Measured fact: the kernel is 100% gather-descriptor-bound (~2.3 ns/desc at
1024-idx dma_gather chunks; DVE fully hidden). So: fewer descriptors.

Sharding: core c owns src nodes [12500c, 12500(c+1)) and the edges whose src
falls there (counts vary ±~300; static caps are maxed over cores, padded
slots discarded by the host). Per core, edges are sorted by (dst_bucket,
src); equal-src runs within a (core, dst_bucket) segment share ONE gathered
src row (~39k runs vs 75k edges). dst rows are gathered per edge (~81k
slots incl. padding). Total ~120k descriptors/core vs 150k for the
all-pairs version.

Layout: runs are length-sorted (desc) and packed 128 to a group; group g
owns l_g = max run length columns of the slot grid. Edge j of the run at
partition p of group g sits at slot (p, group_col0 + j). dst gathers fill
slots column-major; the src row of each run is gathered once into a
resident [128, G, D] tile (indices are hrange-local, so int16-safe with no
src bucketing and tail-free 1024-chunking). DVE multiplies each dst slot
column block by its group's broadcast src column and row-reduces into
scores; the host inverse-maps slots to edges.

h is bf16 (halves SBUF residency; descriptor cost is unchanged).
"""

import ml_dtypes
import numpy as np

import concourse.bacc as bacc
import concourse.mybir as mybir
import concourse.tile as tile

N_CORES = 8
N_NODES = 100000
N_EDGES = 600000
D = 128
P = 128

NODES_PER_CORE = N_NODES // N_CORES   # 12500
RANGE_CAP = 12544                     # hrange rows (>= NODES_PER_CORE)
N_BUCKETS = 4
BUCKET = 25000                        # dst bucket size (int16-safe)
K_GATHER = 1024                       # SWDGE ring-safe chunk
N_QUEUES = 4
DST_BUFS = 2                          # rotating dst gather tiles


def plan(all_src, all_dst):
    """Host layout. Returns (static_plan, per_core_data)."""
    core_of = all_src // NODES_PER_CORE
    per_core = []
    runs_len = [[None] * N_BUCKETS for _ in range(N_CORES)]
    for c in range(N_CORES):
        eidx = np.nonzero(core_of == c)[0]
        src = all_src[eidx]
        dst = all_dst[eidx]
        db = dst // BUCKET
        order = np.lexsort((src, db))          # sort by (dst_bucket, src)
        src, dst, db, eidx = src[order], dst[order], db[order], eidx[order]
        segs = []
        for b in range(N_BUCKETS):
            m = db == b
            sb, dbv, eb = src[m], dst[m], eidx[m]
            # runs of equal src (already sorted by src within bucket)
            uniq, start, cnt = np.unique(sb, return_index=True,
                                         return_counts=True)
            # order runs by length desc (stable)
            ro = np.argsort(-cnt, kind="stable")
            segs.append((uniq[ro], start[ro], cnt[ro], sb, dbv, eb))
            runs_len[c][b] = cnt[ro]
        per_core.append(segs)

    # static caps: per bucket, group count and per-group column capacity
    ell_hat = []
    g_cap = []
    for b in range(N_BUCKETS):
        gmax = max(-(-len(runs_len[c][b]) // P) for c in range(N_CORES))
        g_cap.append(gmax)
        ell = np.zeros(gmax, dtype=np.int64)
        for c in range(N_CORES):
            cnt = runs_len[c][b]
            for g in range(-(-len(cnt) // P)):
                ell[g] = max(ell[g], cnt[g * P])   # desc-sorted: first is max
        ell_hat.append(ell)
    # dst columns per bucket, rounded to chunk granularity (8 cols = 1024)
    dst_cols = [int(-(-int(e.sum()) // 8) * 8) for e in ell_hat]
    src_cols = [int(g) for g in g_cap]            # one src col per group
    return (ell_hat, g_cap, dst_cols, src_cols), per_core


def _chunks(total_idx):
    out = [K_GATHER] * (total_idx // K_GATHER)
    t = total_idx % K_GATHER
    if t:
        out.append(t)                  # already multiple of 128
    return out


def build_program(static_plan, repeats=1, do_dve=True, do_gather=True):
    ell_hat, g_cap, dst_cols, src_cols = static_plan
    G_tot = sum(src_cols)
    C_tot = sum(dst_cols)
    sidx_cols = G_tot * P // 16        # wrapped idx cols
    didx_cols = C_tot * P // 16

    nc = bacc.Bacc("TRN2", target_bir_lowering=False, debug=False,
                   num_swdge_queues=N_QUEUES)
    h = nc.dram_tensor("h", [N_NODES, D], mybir.dt.bfloat16,
                       kind="ExternalInput")
    hrange = nc.dram_tensor("hrange", [RANGE_CAP, D], mybir.dt.bfloat16,
                            kind="ExternalInput")
    sidx = nc.dram_tensor("sidx", [P, sidx_cols], mybir.dt.int16,
                          kind="ExternalInput")
    didx = nc.dram_tensor("didx", [P, didx_cols], mybir.dt.int16,
                          kind="ExternalInput")
    out = nc.dram_tensor("scores", [P, C_tot], mybir.dt.float32,
                         kind="ExternalOutput")

    qn = [0]

    def q():
        qn[0] += 1
        return qn[0] % N_QUEUES

    with tile.TileContext(nc) as tc:
        with (
            tc.tile_pool(name="idxp", bufs=1) as idx_pool,
            tc.tile_pool(name="sr", bufs=1) as src_pool,
            tc.tile_pool(name="sc", bufs=1) as sc_pool,
            tc.tile_pool(name="dp", bufs=DST_BUFS) as dpool,
        ):
            sidx_t = idx_pool.tile([P, sidx_cols], mybir.dt.int16)
            didx_t = idx_pool.tile([P, didx_cols], mybir.dt.int16)
            nc.sync.dma_start(out=sidx_t[:], in_=sidx[:])
            nc.sync.dma_start(out=didx_t[:], in_=didx[:])
            scores = sc_pool.tile([P, C_tot], mybir.dt.float32)
            nc.vector.memset(scores[:], 0.0)
            src_res = src_pool.tile([P, G_tot, D], mybir.dt.bfloat16)

            max_bucket_cols = max(dst_cols)
            for _rep in range(repeats):
                # 1) src rows: one flat gather stream from hrange
                i0, col = 0, 0
                for kk in _chunks(G_tot * P):
                    ct = kk // P
                    S = kk // 16
                    if do_gather:
                        nc.gpsimd.dma_gather(
                            out_ap=src_res[:, col:col + ct, :],
                            in_ap=hrange[:, :],
                            idxs_ap=sidx_t[:, i0:i0 + S],
                            num_idxs=kk, num_idxs_reg=kk,
                            elem_size=D, queue_num=q())
                    i0 += S
                    col += ct
                if not do_gather:
                    nc.gpsimd.memset(src_res[:], 0.0)

                # 2) per dst bucket: gather dst slots, then DVE
                i0 = 0
                bcol0 = 0          # scores column base of bucket
                gcol0 = 0          # src_res column base of bucket
                for b in range(N_BUCKETS):
                    h_b = h[b * BUCKET:(b + 1) * BUCKET, :]
                    cb = dst_cols[b]
                    dtile = dpool.tile([P, max_bucket_cols, D],
                                       mybir.dt.bfloat16, tag="d")
                    col = 0
                    for kk in _chunks(cb * P):
                        ct = kk // P
                        S = kk // 16
                        if do_gather:
                            nc.gpsimd.dma_gather(
                                out_ap=dtile[:, col:col + ct, :],
                                in_ap=h_b,
                                idxs_ap=didx_t[:, i0:i0 + S],
                                num_idxs=kk, num_idxs_reg=kk,
                                elem_size=D, queue_num=q())
                        else:
                            nc.gpsimd.memset(dtile[:, col:col + ct, :], 0.0)
                        i0 += S
                        col += ct
                    if do_dve:
                        # group spans: merge consecutive groups with l==1
                        ell = ell_hat[b]
                        g = 0
                        dcol = 0
                        while g < len(ell):
                            l = int(ell[g])
                            if l == 1:
                                m = len(ell) - g     # desc-sorted: rest are 1
                                d_sl = dtile[:, dcol:dcol + m, :]
                                s_sl = src_res[:, gcol0 + g:gcol0 + g + m, :]
                                nc.vector.tensor_mul(out=d_sl, in0=d_sl,
                                                     in1=s_sl)
                                nc.vector.tensor_reduce(
                                    out=scores[:, bcol0 + dcol:
                                               bcol0 + dcol + m],
                                    in_=d_sl, axis=mybir.AxisListType.X,
                                    op=mybir.AluOpType.add)
                                dcol += m
                                g += m
                            else:
                                d_sl = dtile[:, dcol:dcol + l, :]
                                s_sl = src_res[:, gcol0 + g:gcol0 + g + 1, :]
                                nc.vector.tensor_mul(
                                    out=d_sl, in0=d_sl,
                                    in1=s_sl.to_broadcast([P, l, D]))
                                nc.vector.tensor_reduce(
                                    out=scores[:, bcol0 + dcol:
                                               bcol0 + dcol + l],
                                    in_=d_sl, axis=mybir.AxisListType.X,
                                    op=mybir.AluOpType.add)
                                dcol += l
                                g += 1
                    bcol0 += cb
                    gcol0 += g_cap[b]

            nc.sync.dma_start(out=out[:], in_=scores[:])
    nc.compile()
    return nc


def _wrap_block(flat_i16):
    k = flat_i16.shape[0]
    w = flat_i16.reshape(k // 16, 16).T
    return np.ascontiguousarray(np.tile(w, (8, 1)))


def _wrap_stream(flat_i16):
    segs = []
    off = 0
    for kk in _chunks(flat_i16.shape[0]):
        segs.append(_wrap_block(flat_i16[off:off + kk]))
        off += kk
    return np.concatenate(segs, axis=1)


def make_core_inputs(core_id, segs, static_plan, h):
    ell_hat, g_cap, dst_cols, src_cols = static_plan
    h16 = np.asarray(h, dtype=ml_dtypes.bfloat16)
    lo = core_id * NODES_PER_CORE
    hr = np.zeros((RANGE_CAP, D), dtype=ml_dtypes.bfloat16)
    hi = min(lo + RANGE_CAP, N_NODES)
    hr[:hi - lo] = h16[lo:hi]

    G_tot = sum(g_cap)
    C_tot = sum(dst_cols)
    src_flat = (np.arange(G_tot * P, dtype=np.int64) * 97) % NODES_PER_CORE
    dst_flat = np.empty(C_tot * P, dtype=np.int64)

    e_glob = []          # original edge index per slot-entry
    e_slot_p = []
    e_slot_col = []

    gcol0 = 0
    bcol0 = 0
    for b in range(N_BUCKETS):
        uniq, start, cnt, sb, dbv, eb = segs[b]
        ell = ell_hat[b]
        nr = len(uniq)
        # src slots: run r -> (partition r%P at flat pos g*P + p)
        r = np.arange(nr)
        src_flat[gcol0 * P + r] = uniq - lo
        # dst slots, column-major within bucket
        seg = (np.arange(dst_cols[b] * P, dtype=np.int64) * 89) % BUCKET
        col_of_g = np.concatenate([[0], np.cumsum(ell)])
        for g in range(-(-nr // P)):
            rr = np.arange(g * P, min((g + 1) * P, nr))
            for j in range(int(ell[g])):
                sel = rr[cnt[rr] > j]
                if len(sel) == 0:
                    continue
                p = sel % P
                col = col_of_g[g] + j
                eix = start[sel] + j
                seg[col * P + p] = dbv[eix] - b * BUCKET
                e_glob.append(eb[eix])
                e_slot_p.append(p)
                e_slot_col.append(np.full(len(p), bcol0 + col))
        dst_flat[bcol0 * P:(bcol0 + dst_cols[b]) * P] = seg
        gcol0 += g_cap[b]
        bcol0 += dst_cols[b]

    sidx = _wrap_stream(src_flat.astype(np.int16))
    didx = _wrap_stream(dst_flat.astype(np.int16))
    mapping = (np.concatenate(e_glob), np.concatenate(e_slot_p),
               np.concatenate(e_slot_col))
    return ({"h": h16, "hrange": hr, "sidx": np.ascontiguousarray(sidx),
             "didx": np.ascontiguousarray(didx)}, mapping)


def run(edge_index, h, pad_value=0):
    from concourse.bass_utils import run_bass_kernel_spmd

    h = np.ascontiguousarray(np.asarray(h), dtype=np.float32)
    all_src = np.asarray(edge_index[0], dtype=np.int64)
    all_dst = np.asarray(edge_index[1], dtype=np.int64)
    static_plan, per_core = plan(all_src, all_dst)
    nc = build_program(static_plan)

    in_maps, mappings = [], []
    for c in range(N_CORES):
        m, mapping = make_core_inputs(c, per_core[c], static_plan, h)
        in_maps.append(m)
        mappings.append(mapping)

    res = run_bass_kernel_spmd(nc, in_maps, core_ids=list(range(N_CORES)))

    out = np.empty(N_EDGES, dtype=np.float32)
    for c in range(N_CORES):
        eg, pp, cc = mappings[c]
        scores = res.results[c]["scores"]
        out[eg] = scores[pp, cc]
    return out, res


def kernel(edge_index, h):
    out, _ = run(edge_index, h)
    return out
